# revision 13
# baseline (speedup 1.0000x reference)
"""CTRNN (6 unfolds) Trainium2 Bass kernel, data-parallel over 8 NeuronCores.

Math (per reference):
    w_x = fc_w[:, :512]; w_h = fc_w[:, 512:]
    xw  = x @ w_x^T + b
    repeat 6x:  f_t = tanh(xw + h_t @ w_h^T);  h_{t+1} = 0.9*h_t + 0.1*f_t

Reformulated in pre-activation space so the recurrent matmul can run in
fp8 (DoubleRow, 2x PE throughput) with its quantization error damped 10x:
    z_t := xw + b + h_t @ w_h^T         (pre-activation state)
    f_t  = tanh(z_t)
    z_{t+1} = 0.9*z_t + 0.1*(xw + b) + 0.1*(f_t @ w_h^T)   <- fp8, damped
    h_6  = 0.9^6*h_0 + sum_t 0.1*0.9^(5-t)*f_t             <- on the host

On device the state is kept as z* = 80*z in f32 (f32 is this DVE's fast
path; bf16 is NOT faster) with power-of-two scales folded so each step is
exactly three engine passes per element (the structural floor):
    V: q  = 0.9*z* + xwb          xwb = bf16(8*(xw+b)), scalar_tensor_tensor
    V: z* = q + psum              psum = 8*W@f from e4m3(8*w_h), plain
                                  tensor_tensor over 4-bank [128,2048] PSUM
    S: f8 = tanh(z*/80)           activation scale, fp8e4 out
Each step's f8 streams to DRAM; at t=5 raw z* streams instead and the host
computes f_5 = tanh(z*/80) exactly.  The host does the final
h_6 = 0.9^6*h_0 + sum_t c_t*f_t axpy during gather (h_0 is host-resident).
z*_0 comes from a bf16 matmul (full-scale error path needs > fp8); xw is
bf16.  Elementwise stage (tanh+ship) runs 2 chunk-slots ahead of the
matmul stage, and step-0 tanh is interleaved into the z0 phase.

Device layout: everything transposed ([feature, batch]). Per core: batch
shard of 2048, processed as 4 chunks of 512 (PSUM-bank-sized moving dim).
"""

import numpy as np
from contextlib import ExitStack

import ml_dtypes

import concourse.bass as bass
import concourse.tile as tile
import concourse.mybir as mybir
from concourse.bass_utils import run_bass_kernel_spmd


def _patch_tile_drain():
    """The walrus build in this image encodes at most one sync-wait on a
    Drain CTRL instruction; Tile's kernel-tail drain attaches one wait per
    outstanding proc and fails codegen ("Too many sync wait commands").
    Spread those waits across single-wait SP nops, then emit a bare drain."""
    if getattr(tile.TileContext, "_drain_split_patched", False):
        return
    from concourse.vector_clock import ScopedClock

    def _drain_and_barrier(self, tick_clock, wait_clock):
        nc = self.nc
        collector = nc.sync.nop(nofuse=True)
        wait_clock.add_sem_waits(
            collector.ins, ScopedClock({None: tick_clock.global_clock})
        )
        waits = list(collector.ins.sync_info.on_wait)
        del collector.ins.sync_info.on_wait[1:]
        for w in waits[1:]:
            nop = nc.sync.nop(nofuse=True)
            if nop.ins.sync_info is None:
                nop.ins.sync_info = mybir.SyncInfo(on_wait=[], on_update=[])
            nop.ins.sync_info.on_wait.append(w)
        nc.sync.drain()
        nc.all_engine_barrier()
        assert self.sems is not None
        popped = nc._tile_sem_poison_stack.pop()
        assert popped is self._sem_poison
        nc.clear_and_free_semaphores(list(self.sems.allocated().values()))
        nc.all_engine_barrier()

    tile.TileContext._drain_and_barrier = _drain_and_barrier
    tile.TileContext._drain_split_patched = True


_patch_tile_drain()


def _split_excess_waits_json(bir_json):
    """This image's walrus encodes at most ONE sync-wait per instruction
    (setupSyncWait: "Too many sync wait commands").  Tile attaches as many
    waits as deps require.  Hoist all but one wait of each instruction onto
    injected NoOps, placed just before it on the same engine."""
    import json as _json

    js = _json.loads(bir_json)
    n_split = 0
    for fn in js["functions"]:
        for blk in fn["blocks"]:
            out_insts = []
            for inst in blk["instructions"]:
                si = inst.get("sync_info") or {}
                ow = si.get("on_wait") or []
                if len(ow) > 1:
                    for w in ow[:-1]:
                        n_split += 1
                        nop = {
                            "name": f"I-ws{n_split}",
                            "opcode": "NoOp",
                            "engine": inst["engine"],
                            "ins": [],
                            "outs": [],
                            "sync_info": {"on_update": [], "on_wait": [w]},
                        }
                        if "debug" in inst:
                            nop["debug"] = inst["debug"]
                        out_insts.append(nop)
                    si["on_wait"] = [ow[-1]]
                out_insts.append(inst)
            blk["instructions"] = out_insts
    return _json.dumps(js).encode()


def _patch_compile_for_wait_cap():
    import concourse.bass_utils as _bu

    if getattr(_bu, "_wait_split_patched", False):
        return
    _orig = _bu._compile_bir_impl

    def _impl(bir_json, *args, **kwargs):
        return _orig(_split_excess_waits_json(bir_json), *args, **kwargs)

    _bu._compile_bir_impl = _impl
    _bu._wait_split_patched = True


_patch_compile_for_wait_cap()

B, D_IN, D_H = 16384, 512, 1024
N_CORES = 8
BS = B // N_CORES            # 2048 batch rows per core
UNFOLDS = 6
DT = 0.1
DECAY = 0.9                  # 1 - DT/TAU
CH = 512                     # batch chunk (matmul moving free dim)
NCH = BS // CH               # 4 chunks per core
KB = D_H // 128              # 8 hidden-dim k-blocks
KX = D_IN // 128             # 4 input-dim k-blocks
NU = KB // 2                 # 4 DoubleRow k-block pairs
WSCALE = 8.0                 # wh8 = e4m3(8*w_h^T); psum = 8*W@f
ZSCALE = 80.0                # state z* = 80*z; xwb = 8*(xw+b)
F32 = mybir.dt.float32
BF16 = mybir.dt.bfloat16
F8 = mybir.dt.float8e4
MUL = mybir.AluOpType.mult
ADD = mybir.AluOpType.add
DR = mybir.MatmulPerfMode.DoubleRow


def build_nc() -> bass.Bass:
    nc = bass.Bass()
    x16 = nc.dram_tensor("x16", [D_IN, BS], BF16, kind="ExternalInput")
    h16 = nc.dram_tensor("h16", [D_H, BS], BF16, kind="ExternalInput")
    wx16 = nc.dram_tensor("wx16", [D_IN, D_H], BF16, kind="ExternalInput")
    wh16 = nc.dram_tensor("wh16", [D_H, D_H], BF16, kind="ExternalInput")
    # paired DoubleRow layout: [kpart, (u, p, two, col)] = [128, 8192]
    wh8p = nc.dram_tensor("wh8p", [128, KB * D_H], F8, kind="ExternalInput")
    biasd = nc.dram_tensor("bias", [128, KB], F32, kind="ExternalInput")
    idwd = nc.dram_tensor("idw", [128, 128], BF16, kind="ExternalInput")
    fout = nc.dram_tensor("fout", [(UNFOLDS - 1) * D_H, BS], F8, kind="ExternalOutput")
    zout = nc.dram_tensor("zout", [D_H, BS], F32, kind="ExternalOutput")

    with tile.TileContext(nc) as tc, ExitStack() as ctx:
        persist = ctx.enter_context(tc.tile_pool(name="persist", bufs=1))
        psum_pool = ctx.enter_context(tc.tile_pool(name="psum", bufs=2, space="PSUM"))

        bias_sb = persist.tile([128, KB], F32, name="b_sb", tag="b_sb")
        wh8_sb = persist.tile([128, KB * D_H], F8, name="wh8", tag="wh8")
        xwb = [persist.tile([128, KB * CH], BF16, name=f"xwb{c}", tag=f"xwb{c}")
               for c in range(NCH)]
        zst = [persist.tile([128, KB * CH], F32, name=f"z{c}", tag=f"z{c}")
               for c in range(NCH)]
        f8 = [persist.tile([128, KB * CH], F8, name=f"f8_{c}", tag=f"f8_{c}")
              for c in range(NCH)]
        idw = persist.tile([128, 128], BF16, name="idw", tag="idw")
        zb_pool = ctx.enter_context(tc.tile_pool(name="zb", bufs=3))
        zbtiles = {}

        def stage_act(t, c):
            # f8 = tanh(z*/80) for the next matmuls + ship it to the host.
            # Blocks 6..7's decay is offloaded to the PE: the scalar engine
            # stages zb2 = bf16(0.9*z*[6:8]) (0.9 applied at f32 precision)
            # and an exact-bf16 identity matmul adds it into the psum group.
            if t < UNFOLDS - 1:
                nc.scalar.activation(
                    f8[c][:], zst[c][:], mybir.ActivationFunctionType.Tanh,
                    bias=0.0, scale=float(1.0 / ZSCALE),
                )
                zb = zb_pool.tile([128, 2 * CH], BF16, name="zb", tag="zb")
                nc.scalar.mul(zb[:], zst[c][:, 6 * CH:8 * CH], float(DECAY))
                zbtiles[(t, c)] = zb
                for jb in range(KB):
                    nc.sync.dma_start(
                        out=fout[t * D_H + jb * 128: t * D_H + (jb + 1) * 128,
                                 c * CH:(c + 1) * CH],
                        in_=f8[c][:, jb * CH:(jb + 1) * CH],
                    )
            else:
                for jb in range(KB):
                    nc.sync.dma_start(
                        out=zout[jb * 128:(jb + 1) * 128,
                                 c * CH:(c + 1) * CH],
                        in_=zst[c][:, jb * CH:(jb + 1) * CH],
                    )

        def stage_decay(t, c):
            # in-place z* <- 0.9*z* + xwb for blocks 0..5, AFTER act(t,c)
            # has read z* (blocks 6..7 decay via the PE identity matmul)
            nc.vector.scalar_tensor_tensor(
                zst[c][:, 0:6 * CH], zst[c][:, 0:6 * CH], float(DECAY),
                xwb[c][:, 0:6 * CH], op0=MUL, op1=ADD,
            )

        def stage_b(t, c):
            if t >= UNFOLDS - 1:
                return
            if t > 0:
                stage_decay(t, c)
            zb = zbtiles.pop((t, c))
            for pp in range(KB // 4):
                ps = psum_pool.tile([128, 4 * CH], F32, name="ps", tag="ps")
                for qr in range(4):
                    p = 4 * pp + qr
                    if p >= 6:
                        nc.tensor.matmul(
                            ps[:, qr * CH:(qr + 1) * CH],
                            idw[:, :],
                            zb[:, (p - 6) * CH:(p - 5) * CH],
                            start=True, stop=False,
                            skip_group_check=True,
                        )
                    for u in range(NU):
                        off = (u * KB + p) * 256
                        nc.tensor.matmul(
                            ps[:, qr * CH:(qr + 1) * CH],
                            wh8_sb[:, off:off + 256].rearrange(
                                "q (two m) -> q two m", two=2),
                            f8[c][:, (2 * u) * CH:(2 * u + 2) * CH].rearrange(
                                "q (two n) -> q two n", two=2),
                            start=(u == 0 and p < 6),
                            stop=(u == NU - 1),
                            perf_mode=DR,
                            skip_group_check=True,
                        )
                if pp == 0:
                    nc.vector.tensor_tensor(
                        zst[c][:, 0:4 * CH],
                        zst[c][:, 0:4 * CH], ps[:], op=ADD,
                    )
                else:
                    nc.vector.tensor_tensor(
                        zst[c][:, 4 * CH:6 * CH],
                        zst[c][:, 4 * CH:6 * CH], ps[:, 0:2 * CH], op=ADD,
                    )
                    nc.vector.tensor_tensor(
                        zst[c][:, 6 * CH:8 * CH],
                        xwb[c][:, 6 * CH:8 * CH], ps[:, 2 * CH:4 * CH], op=ADD,
                    )

        slots = [(t, c) for t in range(UNFOLDS) for c in range(NCH)]
        # stage_act(0,0)/(0,1)/(0,2)/(0,3) are emitted inside the y0 loop

        with tc.tile_pool(name="pre", bufs=1) as pre:
            wx_sb = pre.tile([128, KX * D_H], BF16, name="wx_sb", tag="wx_sb")
            x_sb = [pre.tile([128, KX * CH], BF16, name="x_sb", tag=f"x_sb{c}")
                    for c in range(NCH)]
            wh16_sb = pre.tile([128, KB * D_H], BF16, name="wh16", tag="wh16")
            h16_sb = [pre.tile([128, KB * CH], BF16, name="h16", tag=f"h16_{c}")
                      for c in range(NCH)]

            nc.sync.dma_start(out=bias_sb[:], in_=biasd[:, :])
            # head-critical loads first: wx + x chunk 0 gate the first matmul
            for kb in range(KX):
                nc.gpsimd.dma_start(
                    out=wx_sb[:, kb * D_H:(kb + 1) * D_H],
                    in_=wx16[kb * 128:(kb + 1) * 128, :],
                )
                nc.gpsimd.dma_start(
                    out=x_sb[0][:, kb * CH:(kb + 1) * CH],
                    in_=x16[kb * 128:(kb + 1) * 128, 0:CH],
                )
            for c in range(1, NCH):
                nc.gpsimd.dma_start(
                    out=x_sb[c][:].rearrange("q (kb n) -> q kb n", n=CH),
                    in_=x16[:, c * CH:(c + 1) * CH].rearrange(
                        "(kb q) n -> q kb n", q=128),
                )
            # y0-phase + step-phase loads; they have all of phase 1 to land
            nc.gpsimd.dma_start(
                out=wh16_sb[:].rearrange("q (jb m) -> q jb m", m=D_H),
                in_=wh16[:, :].rearrange("(jb q) m -> q jb m", q=128),
            )
            for c in range(NCH):
                nc.gpsimd.dma_start(
                    out=h16_sb[c][:].rearrange("q (jb n) -> q jb n", n=CH),
                    in_=h16[:, c * CH:(c + 1) * CH].rearrange(
                        "(jb q) n -> q jb n", q=128),
                )
            nc.gpsimd.dma_start(out=wh8_sb[:], in_=wh8p[:, :])
            nc.gpsimd.dma_start(out=idw[:], in_=idwd[:, :])

            # --- phase 1: xwb = bf16(8*(x @ w_x^T + b)), bf16 matmul ---
            for c in range(NCH):
                for pp in range(KB // 4):
                    ps = psum_pool.tile([128, 4 * CH], F32, name="ps", tag="ps")
                    for qr in range(4):
                        p = 4 * pp + qr
                        for kb in range(KX):
                            nc.tensor.matmul(
                                ps[:, qr * CH:(qr + 1) * CH],
                                wx_sb[:, kb * D_H + p * 128: kb * D_H + (p + 1) * 128],
                                x_sb[c][:, kb * CH:(kb + 1) * CH],
                                start=(kb == 0),
                                stop=(kb == KX - 1),
                            )
                    for qr in range(4):
                        p = 4 * pp + qr
                        nc.scalar.activation(
                            xwb[c][:, p * CH:(p + 1) * CH],
                            ps[:, qr * CH:(qr + 1) * CH],
                            mybir.ActivationFunctionType.Identity,
                            bias=bias_sb[:, p:p + 1], scale=float(WSCALE),
                        )

            # --- phase 2: z*_0 = 10*(xw + b + w_h @ h_0), bf16 matmul ---
            # State z* = 10*z in f32 (f32 is the DVE fast path; bf16 is not
            # faster on this DVE).  f = tanh(0.1*z*) via the act scale.
            for c in range(NCH):
                for pp in range(KB // 4):
                    ps = psum_pool.tile([128, 4 * CH], F32, name="ps", tag="ps")
                    for qr in range(4):
                        p = 4 * pp + qr
                        for jb in range(KB):
                            nc.tensor.matmul(
                                ps[:, qr * CH:(qr + 1) * CH],
                                wh16_sb[:, jb * D_H + p * 128: jb * D_H + (p + 1) * 128],
                                h16_sb[c][:, jb * CH:(jb + 1) * CH],
                                start=(jb == 0),
                                stop=(jb == KB - 1),
                            )
                    nc.vector.tensor_scalar_mul(
                        zst[c][:, 4 * pp * CH:(4 * pp + 4) * CH], ps[:],
                        float(ZSCALE))
                nc.vector.scalar_tensor_tensor(
                    zst[c][:], xwb[c][:], 10.0, zst[c][:], op0=MUL, op1=ADD,
                )
                if c >= 1:
                    # step-0 tanh for chunk c-1 overlaps chunk c+1's y0
                    # matmuls; the in-place decay trails one more chunk so
                    # the V queue never stalls waiting on an act
                    stage_act(0, c - 1)
                if c >= 2:
                    stage_decay(0, c - 2)
            stage_act(0, NCH - 1)
            stage_decay(0, NCH - 2)
            stage_decay(0, NCH - 1)

        # --- phase 3: unfold loop (fp8 DoubleRow recurrent matmuls) ---
        # Per step: f8 = tanh(z*/80)       (scalar engine, fp8 out; t<5)
        #           Q  = 0.9*z* + xwb      (stt, SBUF-only, overlaps MMs)
        #           z* = Q + psum          (tensor_tensor over 4-bank psum)
        # f8 ships to DRAM each step; at t=5 raw z* ships instead and the
        # host computes f_5 = tanh(z*/80) exactly (no fp8 rounding).
        # stage_act(t,c) runs 2 chunk-slots ahead of stage_b(t,c).
        for i, (t, c) in enumerate(slots):
            stage_b(t, c)
            if NCH <= i + 2 < len(slots):
                stage_act(*slots[i + 2])
    return nc


_NC_CACHE = {}


def _get_nc() -> bass.Bass:
    if "nc" not in _NC_CACHE:
        _NC_CACHE["nc"] = build_nc()
    return _NC_CACHE["nc"]


def make_in_maps(x, h, fc_w, fc_b):
    x = np.asarray(x, dtype=np.float32)
    h = np.asarray(h, dtype=np.float32)
    fc_w = np.asarray(fc_w, dtype=np.float32)
    fc_b = np.asarray(fc_b, dtype=np.float32)
    xT = np.ascontiguousarray(x.T).astype(ml_dtypes.bfloat16)      # [D_IN, B]
    hT = np.ascontiguousarray(h.T).astype(ml_dtypes.bfloat16)      # [D_H, B]
    wx16 = np.ascontiguousarray(fc_w[:, :D_IN].T).astype(ml_dtypes.bfloat16)
    whT = np.ascontiguousarray(fc_w[:, D_IN:].T)                   # [D_H, D_H]
    wh16 = whT.astype(ml_dtypes.bfloat16)
    # DoubleRow-paired fp8 weights: [k, h] -> [kpart, (u, p, two, col)]
    w8 = (WSCALE * whT).astype(ml_dtypes.float8_e4m3)
    wh8p = np.ascontiguousarray(
        w8.reshape(NU, 2, 128, KB, 128).transpose(2, 0, 3, 1, 4)
        .reshape(128, KB * D_H))
    bias = np.ascontiguousarray(WSCALE * fc_b.reshape(KB, 128).T)  # [128, KB]
    in_maps = []
    for i in range(N_CORES):
        sl = slice(i * BS, (i + 1) * BS)
        in_maps.append({
            "x16": np.ascontiguousarray(xT[:, sl]),
            "h16": np.ascontiguousarray(hT[:, sl]),
            "wx16": wx16,
            "wh16": wh16,
            "wh8p": wh8p,
            "bias": bias,
            "idw": np.eye(128, dtype=ml_dtypes.bfloat16),
        })
    return in_maps


def gather_out(results, h):
    # device streams f8_t (t=0..4) and the raw final pre-activation z*_5;
    # finish h_6 = 0.9^6 h_0 + sum_t 0.1*0.9^(5-t) f_t here on the host
    fT = np.concatenate([results[i]["fout"] for i in range(N_CORES)], axis=1)
    zT = np.concatenate([results[i]["zout"] for i in range(N_CORES)], axis=1)
    acc = np.zeros((D_H, B), dtype=np.float32)
    for t in range(UNFOLDS - 1):
        ct = DT * DECAY ** (UNFOLDS - 1 - t)
        acc += ct * fT[t * D_H:(t + 1) * D_H].astype(np.float32)
    acc += DT * np.tanh(zT.astype(np.float32) / ZSCALE)
    out = (DECAY ** UNFOLDS) * np.asarray(h, dtype=np.float32) + acc.T
    return np.ascontiguousarray(out)                               # [B, D_H]


def kernel(x, h, fc_w, fc_b):
    nc = _get_nc()
    in_maps = make_in_maps(x, h, fc_w, fc_b)
    res = run_bass_kernel_spmd(nc, in_maps, list(range(N_CORES)))
    out = gather_out(res.results, h)
    return (out, out)


if __name__ == "__main__":
    rng = np.random.default_rng(0)
    x = rng.standard_normal((B, D_IN), dtype=np.float32)
    h = rng.standard_normal((B, D_H), dtype=np.float32)
    fc_w = rng.standard_normal((D_H, D_IN + D_H), dtype=np.float32) / np.sqrt(D_IN + D_H)
    fc_b = np.zeros((D_H,), dtype=np.float32)
    o, _ = kernel(x, h, fc_w, fc_b)
    print(o.shape, o.dtype)


# revision 14
# speedup vs baseline: 1.1137x; 1.1137x over previous
"""CTRNN (6 unfolds) Trainium2 Bass kernel, data-parallel over 8 NeuronCores.

Math (per reference):
    w_x = fc_w[:, :512]; w_h = fc_w[:, 512:]
    xw  = x @ w_x^T + b
    repeat 6x:  f_t = tanh(xw + h_t @ w_h^T);  h_{t+1} = 0.9*h_t + 0.1*f_t

Reformulated in pre-activation space so the recurrent matmul can run in
fp8 (DoubleRow, 2x PE throughput) with its quantization error damped 10x:
    z_t := xw + b + h_t @ w_h^T         (pre-activation state)
    f_t  = tanh(z_t)
    z_{t+1} = 0.9*z_t + 0.1*(xw + b) + 0.1*(f_t @ w_h^T)   <- fp8, damped
    h_6  = 0.9^6*h_0 + sum_t 0.1*0.9^(5-t)*f_t             <- on the host

On device the state is kept as z* = 80*z in f32 (f32 is this DVE's fast
path; bf16 is NOT faster) with power-of-two scales folded so each step is
exactly three engine passes per element (the structural floor):
    V: q  = 0.9*z* + xwb          xwb = bf16(8*(xw+b)), scalar_tensor_tensor
    V: z* = q + psum              psum = 8*W@f from e4m3(8*w_h), plain
                                  tensor_tensor over 4-bank [128,2048] PSUM
    S: f8 = tanh(z*/80)           activation scale, fp8e4 out
Each step's f8 streams to DRAM; at t=5 raw z* streams instead and the host
computes f_5 = tanh(z*/80) exactly.  The host does the final
h_6 = 0.9^6*h_0 + sum_t c_t*f_t axpy during gather (h_0 is host-resident).
z*_0 comes from a bf16 matmul (full-scale error path needs > fp8); xw is
bf16.  Elementwise stage (tanh+ship) runs 2 chunk-slots ahead of the
matmul stage, and step-0 tanh is interleaved into the z0 phase.

Device layout: everything transposed ([feature, batch]). Per core: batch
shard of 2048, processed as 4 chunks of 512 (PSUM-bank-sized moving dim).
"""

import numpy as np
from contextlib import ExitStack

import ml_dtypes

import concourse.bass as bass
import concourse.tile as tile
import concourse.mybir as mybir
from concourse.bass_utils import run_bass_kernel_spmd


def _patch_tile_drain():
    """The walrus build in this image encodes at most one sync-wait on a
    Drain CTRL instruction; Tile's kernel-tail drain attaches one wait per
    outstanding proc and fails codegen ("Too many sync wait commands").
    Spread those waits across single-wait SP nops, then emit a bare drain."""
    if getattr(tile.TileContext, "_drain_split_patched", False):
        return
    from concourse.vector_clock import ScopedClock

    def _drain_and_barrier(self, tick_clock, wait_clock):
        nc = self.nc
        collector = nc.sync.nop(nofuse=True)
        wait_clock.add_sem_waits(
            collector.ins, ScopedClock({None: tick_clock.global_clock})
        )
        waits = list(collector.ins.sync_info.on_wait)
        del collector.ins.sync_info.on_wait[1:]
        for w in waits[1:]:
            nop = nc.sync.nop(nofuse=True)
            if nop.ins.sync_info is None:
                nop.ins.sync_info = mybir.SyncInfo(on_wait=[], on_update=[])
            nop.ins.sync_info.on_wait.append(w)
        nc.sync.drain()
        nc.all_engine_barrier()
        assert self.sems is not None
        popped = nc._tile_sem_poison_stack.pop()
        assert popped is self._sem_poison
        nc.clear_and_free_semaphores(list(self.sems.allocated().values()))
        nc.all_engine_barrier()

    tile.TileContext._drain_and_barrier = _drain_and_barrier
    tile.TileContext._drain_split_patched = True


_patch_tile_drain()


def _split_excess_waits_json(bir_json):
    """This image's walrus encodes at most ONE sync-wait per instruction
    (setupSyncWait: "Too many sync wait commands").  Tile attaches as many
    waits as deps require.  Hoist all but one wait of each instruction onto
    injected NoOps, placed just before it on the same engine."""
    import json as _json

    js = _json.loads(bir_json)
    n_split = 0
    for fn in js["functions"]:
        for blk in fn["blocks"]:
            out_insts = []
            for inst in blk["instructions"]:
                si = inst.get("sync_info") or {}
                ow = si.get("on_wait") or []
                if len(ow) > 1:
                    for w in ow[:-1]:
                        n_split += 1
                        nop = {
                            "name": f"I-ws{n_split}",
                            "opcode": "NoOp",
                            "engine": inst["engine"],
                            "ins": [],
                            "outs": [],
                            "sync_info": {"on_update": [], "on_wait": [w]},
                        }
                        if "debug" in inst:
                            nop["debug"] = inst["debug"]
                        out_insts.append(nop)
                    si["on_wait"] = [ow[-1]]
                out_insts.append(inst)
            blk["instructions"] = out_insts
    return _json.dumps(js).encode()


def _patch_compile_for_wait_cap():
    import concourse.bass_utils as _bu

    if getattr(_bu, "_wait_split_patched", False):
        return
    _orig = _bu._compile_bir_impl

    def _impl(bir_json, *args, **kwargs):
        return _orig(_split_excess_waits_json(bir_json), *args, **kwargs)

    _bu._compile_bir_impl = _impl
    _bu._wait_split_patched = True


_patch_compile_for_wait_cap()

B, D_IN, D_H = 16384, 512, 1024
N_CORES = 8
BS = B // N_CORES            # 2048 batch rows per core
UNFOLDS = 6
DT = 0.1
DECAY = 0.9                  # 1 - DT/TAU
CH = 512                     # batch chunk (matmul moving free dim)
NCH = BS // CH               # 4 chunks per core
KB = D_H // 128              # 8 hidden-dim k-blocks
KX = D_IN // 128             # 4 input-dim k-blocks
NU = KB // 2                 # 4 DoubleRow k-block pairs
WSCALE = 8.0                 # wh8 = e4m3(8*w_h^T); psum = 8*W@f
ZSCALE = 80.0                # state z* = 80*z; xwb = 8*(xw+b)
F32 = mybir.dt.float32
BF16 = mybir.dt.bfloat16
F8 = mybir.dt.float8e4
MUL = mybir.AluOpType.mult
ADD = mybir.AluOpType.add
DR = mybir.MatmulPerfMode.DoubleRow


def build_nc() -> bass.Bass:
    nc = bass.Bass()
    x16 = nc.dram_tensor("x16", [D_IN, BS], BF16, kind="ExternalInput")
    h16 = nc.dram_tensor("h16", [D_H, BS], BF16, kind="ExternalInput")
    wx16 = nc.dram_tensor("wx16", [D_IN, D_H], BF16, kind="ExternalInput")
    wh16 = nc.dram_tensor("wh16", [D_H, D_H], BF16, kind="ExternalInput")
    # paired DoubleRow layout: [kpart, (u, p, two, col)] = [128, 8192]
    wh8p = nc.dram_tensor("wh8p", [128, KB * D_H], F8, kind="ExternalInput")
    biasd = nc.dram_tensor("bias", [128, KB], F32, kind="ExternalInput")
    fout = nc.dram_tensor("fout", [(UNFOLDS - 1) * D_H, BS], F8, kind="ExternalOutput")
    zout = nc.dram_tensor("zout", [D_H, BS], F32, kind="ExternalOutput")

    with tile.TileContext(nc) as tc, ExitStack() as ctx:
        persist = ctx.enter_context(tc.tile_pool(name="persist", bufs=1))
        psum_pool = ctx.enter_context(tc.tile_pool(name="psum", bufs=2, space="PSUM"))

        bias_sb = persist.tile([128, KB], F32, name="b_sb", tag="b_sb")
        wh8_sb = persist.tile([128, KB * D_H], F8, name="wh8", tag="wh8")
        xwb = [persist.tile([128, KB * CH], BF16, name=f"xwb{c}", tag=f"xwb{c}")
               for c in range(NCH)]
        zst = [persist.tile([128, KB * CH], F32, name=f"z{c}", tag=f"z{c}")
               for c in range(NCH)]
        f8 = [persist.tile([128, KB * CH], F8, name=f"f8_{c}", tag=f"f8_{c}")
              for c in range(NCH)]

        def stage_act(t, c):
            # f8 = tanh(z*/80) for the next matmuls + ship it to the host
            if t < UNFOLDS - 1:
                nc.scalar.activation(
                    f8[c][:], zst[c][:], mybir.ActivationFunctionType.Tanh,
                    bias=0.0, scale=float(1.0 / ZSCALE),
                )
                for jb in range(KB):
                    nc.sync.dma_start(
                        out=fout[t * D_H + jb * 128: t * D_H + (jb + 1) * 128,
                                 c * CH:(c + 1) * CH],
                        in_=f8[c][:, jb * CH:(jb + 1) * CH],
                    )
            else:
                for jb in range(KB):
                    nc.sync.dma_start(
                        out=zout[jb * 128:(jb + 1) * 128,
                                 c * CH:(c + 1) * CH],
                        in_=zst[c][:, jb * CH:(jb + 1) * CH],
                    )

        def stage_decay(t, c):
            # in-place z* <- 0.9*z* + xwb, AFTER act(t,c) has read z*
            nc.vector.scalar_tensor_tensor(
                zst[c][:], zst[c][:], float(DECAY), xwb[c][:], op0=MUL, op1=ADD,
            )

        def stage_b(t, c):
            if t >= UNFOLDS - 1:
                return
            if t > 0:
                stage_decay(t, c)
            for pp in range(KB // 4):
                ps = psum_pool.tile([128, 4 * CH], F32, name="ps", tag="ps")
                for qr in range(4):
                    p = 4 * pp + qr
                    for u in range(NU):
                        off = (u * KB + p) * 256
                        nc.tensor.matmul(
                            ps[:, qr * CH:(qr + 1) * CH],
                            wh8_sb[:, off:off + 256].rearrange(
                                "q (two m) -> q two m", two=2),
                            f8[c][:, (2 * u) * CH:(2 * u + 2) * CH].rearrange(
                                "q (two n) -> q two n", two=2),
                            start=(u == 0),
                            stop=(u == NU - 1),
                            perf_mode=DR,
                        )
                nc.vector.tensor_tensor(
                    zst[c][:, 4 * pp * CH:(4 * pp + 4) * CH],
                    zst[c][:, 4 * pp * CH:(4 * pp + 4) * CH], ps[:], op=ADD,
                )

        slots = [(t, c) for t in range(UNFOLDS) for c in range(NCH)]
        # stage_act(0,0)/(0,1)/(0,2)/(0,3) are emitted inside the y0 loop

        with tc.tile_pool(name="pre", bufs=1) as pre:
            wx_sb = pre.tile([128, KX * D_H], BF16, name="wx_sb", tag="wx_sb")
            x_sb = [pre.tile([128, KX * CH], BF16, name="x_sb", tag=f"x_sb{c}")
                    for c in range(NCH)]
            wh16_sb = pre.tile([128, KB * D_H], BF16, name="wh16", tag="wh16")
            h16_sb = [pre.tile([128, KB * CH], BF16, name="h16", tag=f"h16_{c}")
                      for c in range(NCH)]

            nc.sync.dma_start(out=bias_sb[:], in_=biasd[:, :])
            # head-critical loads first: wx + x chunk 0 gate the first matmul
            for kb in range(KX):
                nc.gpsimd.dma_start(
                    out=wx_sb[:, kb * D_H:(kb + 1) * D_H],
                    in_=wx16[kb * 128:(kb + 1) * 128, :],
                )
                nc.gpsimd.dma_start(
                    out=x_sb[0][:, kb * CH:(kb + 1) * CH],
                    in_=x16[kb * 128:(kb + 1) * 128, 0:CH],
                )
            for c in range(1, NCH):
                nc.gpsimd.dma_start(
                    out=x_sb[c][:].rearrange("q (kb n) -> q kb n", n=CH),
                    in_=x16[:, c * CH:(c + 1) * CH].rearrange(
                        "(kb q) n -> q kb n", q=128),
                )
            # y0-phase + step-phase loads; they have all of phase 1 to land
            nc.gpsimd.dma_start(
                out=wh16_sb[:].rearrange("q (jb m) -> q jb m", m=D_H),
                in_=wh16[:, :].rearrange("(jb q) m -> q jb m", q=128),
            )
            for c in range(NCH):
                nc.gpsimd.dma_start(
                    out=h16_sb[c][:].rearrange("q (jb n) -> q jb n", n=CH),
                    in_=h16[:, c * CH:(c + 1) * CH].rearrange(
                        "(jb q) n -> q jb n", q=128),
                )
            nc.gpsimd.dma_start(out=wh8_sb[:], in_=wh8p[:, :])

            # --- phase 1: xwb = bf16(8*(x @ w_x^T + b)), bf16 matmul ---
            for c in range(NCH):
                for pp in range(KB // 4):
                    ps = psum_pool.tile([128, 4 * CH], F32, name="ps", tag="ps")
                    for qr in range(4):
                        p = 4 * pp + qr
                        for kb in range(KX):
                            nc.tensor.matmul(
                                ps[:, qr * CH:(qr + 1) * CH],
                                wx_sb[:, kb * D_H + p * 128: kb * D_H + (p + 1) * 128],
                                x_sb[c][:, kb * CH:(kb + 1) * CH],
                                start=(kb == 0),
                                stop=(kb == KX - 1),
                            )
                    for qr in range(4):
                        p = 4 * pp + qr
                        nc.scalar.activation(
                            xwb[c][:, p * CH:(p + 1) * CH],
                            ps[:, qr * CH:(qr + 1) * CH],
                            mybir.ActivationFunctionType.Identity,
                            bias=bias_sb[:, p:p + 1], scale=float(WSCALE),
                        )

            # --- phase 2: z*_0 = 10*(xw + b + w_h @ h_0), bf16 matmul ---
            # State z* = 10*z in f32 (f32 is the DVE fast path; bf16 is not
            # faster on this DVE).  f = tanh(0.1*z*) via the act scale.
            for c in range(NCH):
                for pp in range(KB // 4):
                    ps = psum_pool.tile([128, 4 * CH], F32, name="ps", tag="ps")
                    for qr in range(4):
                        p = 4 * pp + qr
                        for jb in range(KB):
                            nc.tensor.matmul(
                                ps[:, qr * CH:(qr + 1) * CH],
                                wh16_sb[:, jb * D_H + p * 128: jb * D_H + (p + 1) * 128],
                                h16_sb[c][:, jb * CH:(jb + 1) * CH],
                                start=(jb == 0),
                                stop=(jb == KB - 1),
                            )
                    nc.vector.tensor_scalar_mul(
                        zst[c][:, 4 * pp * CH:(4 * pp + 4) * CH], ps[:],
                        float(ZSCALE))
                nc.vector.scalar_tensor_tensor(
                    zst[c][:], xwb[c][:], 10.0, zst[c][:], op0=MUL, op1=ADD,
                )
                if c >= 1:
                    # step-0 tanh for chunk c-1 overlaps chunk c+1's y0
                    # matmuls; the in-place decay trails one more chunk so
                    # the V queue never stalls waiting on an act
                    stage_act(0, c - 1)
                if c >= 2:
                    stage_decay(0, c - 2)
            stage_act(0, NCH - 1)
            stage_decay(0, NCH - 2)
            stage_decay(0, NCH - 1)

        # --- phase 3: unfold loop (fp8 DoubleRow recurrent matmuls) ---
        # Per step: f8 = tanh(z*/80)       (scalar engine, fp8 out; t<5)
        #           Q  = 0.9*z* + xwb      (stt, SBUF-only, overlaps MMs)
        #           z* = Q + psum          (tensor_tensor over 4-bank psum)
        # f8 ships to DRAM each step; at t=5 raw z* ships instead and the
        # host computes f_5 = tanh(z*/80) exactly (no fp8 rounding).
        # stage_act(t,c) runs 2 chunk-slots ahead of stage_b(t,c).
        for i, (t, c) in enumerate(slots):
            stage_b(t, c)
            if NCH <= i + 2 < len(slots):
                stage_act(*slots[i + 2])
    return nc


_NC_CACHE = {}


def _get_nc() -> bass.Bass:
    if "nc" not in _NC_CACHE:
        _NC_CACHE["nc"] = build_nc()
    return _NC_CACHE["nc"]


def make_in_maps(x, h, fc_w, fc_b):
    x = np.asarray(x, dtype=np.float32)
    h = np.asarray(h, dtype=np.float32)
    fc_w = np.asarray(fc_w, dtype=np.float32)
    fc_b = np.asarray(fc_b, dtype=np.float32)
    xT = np.ascontiguousarray(x.T).astype(ml_dtypes.bfloat16)      # [D_IN, B]
    hT = np.ascontiguousarray(h.T).astype(ml_dtypes.bfloat16)      # [D_H, B]
    wx16 = np.ascontiguousarray(fc_w[:, :D_IN].T).astype(ml_dtypes.bfloat16)
    whT = np.ascontiguousarray(fc_w[:, D_IN:].T)                   # [D_H, D_H]
    wh16 = whT.astype(ml_dtypes.bfloat16)
    # DoubleRow-paired fp8 weights: [k, h] -> [kpart, (u, p, two, col)]
    w8 = (WSCALE * whT).astype(ml_dtypes.float8_e4m3)
    wh8p = np.ascontiguousarray(
        w8.reshape(NU, 2, 128, KB, 128).transpose(2, 0, 3, 1, 4)
        .reshape(128, KB * D_H))
    bias = np.ascontiguousarray(WSCALE * fc_b.reshape(KB, 128).T)  # [128, KB]
    in_maps = []
    for i in range(N_CORES):
        sl = slice(i * BS, (i + 1) * BS)
        in_maps.append({
            "x16": np.ascontiguousarray(xT[:, sl]),
            "h16": np.ascontiguousarray(hT[:, sl]),
            "wx16": wx16,
            "wh16": wh16,
            "wh8p": wh8p,
            "bias": bias,
        })
    return in_maps


def gather_out(results, h):
    # device streams f8_t (t=0..4) and the raw final pre-activation z*_5;
    # finish h_6 = 0.9^6 h_0 + sum_t 0.1*0.9^(5-t) f_t here on the host
    fT = np.concatenate([results[i]["fout"] for i in range(N_CORES)], axis=1)
    zT = np.concatenate([results[i]["zout"] for i in range(N_CORES)], axis=1)
    acc = np.zeros((D_H, B), dtype=np.float32)
    for t in range(UNFOLDS - 1):
        ct = DT * DECAY ** (UNFOLDS - 1 - t)
        acc += ct * fT[t * D_H:(t + 1) * D_H].astype(np.float32)
    acc += DT * np.tanh(zT.astype(np.float32) / ZSCALE)
    out = (DECAY ** UNFOLDS) * np.asarray(h, dtype=np.float32) + acc.T
    return np.ascontiguousarray(out)                               # [B, D_H]


def kernel(x, h, fc_w, fc_b):
    nc = _get_nc()
    in_maps = make_in_maps(x, h, fc_w, fc_b)
    res = run_bass_kernel_spmd(nc, in_maps, list(range(N_CORES)))
    out = gather_out(res.results, h)
    return (out, out)


if __name__ == "__main__":
    rng = np.random.default_rng(0)
    x = rng.standard_normal((B, D_IN), dtype=np.float32)
    h = rng.standard_normal((B, D_H), dtype=np.float32)
    fc_w = rng.standard_normal((D_H, D_IN + D_H), dtype=np.float32) / np.sqrt(D_IN + D_H)
    fc_b = np.zeros((D_H,), dtype=np.float32)
    o, _ = kernel(x, h, fc_w, fc_b)
    print(o.shape, o.dtype)
